# revision 11
# baseline (speedup 1.0000x reference)
"""Causal self-attention (B=2, T=2048, C=1024, H=16, D=64) on 8 trn2 cores.

Sharding: tensor-parallel over (batch, head-group). Core c handles batch
c//4 and heads 4*(c%4) .. 4*(c%4)+4: its 4 heads' QKV projection, causal
attention, and the partial output projection (W_proj row-shard). The 4
partials per batch are summed on the host (the Megatron all-reduce at
gather time), which also adds b_proj.

v3 design notes (what matters on this part):
  - The PE clock demotes to half speed on any idle gap and takes ~3us of
    gapless work to recover, so the PE instruction stream is explicitly
    software-pipelined: S-score matmuls for a PAIR of heads run 2 groups
    ahead of their exp-gated P@V matmuls, and QKV for round t4+1 plus
    the output projection for round t4-1 are spliced into the stream as
    filler between dependent groups.
  - Score path in fp8 (e4m3) with DoubleRow: x^T and wq/wk are
    pre-quantized on the host (weights scaled by 32 to sit in e4m3
    normal range; the 32*32 folds into the exp scale 1/8192). Q^T/K^T
    live as [64, 2, T] f8 tiles (heads 2m,2m+1 at partition bases 0/32,
    j=d//32) so S^T runs fp8 DoubleRow with 2x32 contraction.
  - Value path in f16 end-to-end (x^T f16, wv f16, V f16, P=exp in f16,
    P@V f16, proj f16): fp8 there costs ~2-3% output error (peaked
    softmax does not average it away), over the 2e-2 gate.
  - V' carries 64 duplicated ones-columns so P@V' accumulates the
    softmax denominator in PSUM rows 64:127; normalize is
    copy/recip/multiply on DVE (reciprocal_approx_fast misreads PSUM,
    so the denominator is copied to SBUF first).
  - Diagonal-block causal masks multiply exp output on the Pool engine
    (2 blocks batched per op); DVE carries only the PSUM-touching work.
  - y^T partials leave as f32 straight from PSUM via DMA (no engine
    downcast); host sums 4 partials per batch and adds b_proj.
"""
import os
import sys
import numpy as np

B, T, C = 2, 2048, 1024
H, D = 16, 64
HPC = 4                 # heads per core
QC = HPC * D            # 256 qkv cols per core
NCORES = 8
NT = T // 128           # 16 k-chunks of 128
NT4 = T // 512          # 4 q-chunks of 512
SCALE = 1.0 / np.sqrt(D)
WS = 32.0               # fp8 pre-scale on wq/wk + biases
EXP_SCALE = SCALE / (WS * WS)   # = 1/8192

_cache = {}


def _ensure_env():
    for p in ("/opt/trn_rl_repo", "/root/.axon_site/_ro/trn_rl_repo"):
        if os.path.isdir(p) and p not in sys.path:
            sys.path.append(p)
    jp = os.environ.get("JAX_PLATFORMS")
    if jp and "axon" not in jp and "jax" not in sys.modules:
        os.environ["JAX_PLATFORMS"] = ""


def _build():
    import concourse.bass as bass
    import concourse.bacc as bacc
    import concourse.mybir as mybir
    import concourse.tile as tile

    F32 = mybir.dt.float32
    F16 = mybir.dt.float16
    F8 = mybir.dt.float8e4
    AF = mybir.ActivationFunctionType
    DR = mybir.MatmulPerfMode.DoubleRow
    MUL = bass.mybir.AluOpType.mult
    ADD = bass.mybir.AluOpType.add

    nc = bacc.Bacc()
    xq_d = nc.dram_tensor("xq", [128, 8 * T], F8, kind="ExternalInput")
    xv_d = nc.dram_tensor("xv", [128, 8 * T], F16, kind="ExternalInput")
    wq_d = nc.dram_tensor("wq", [128, 8 * QC], F8, kind="ExternalInput")
    wk_d = nc.dram_tensor("wk", [128, 8 * QC], F8, kind="ExternalInput")
    wv_d = nc.dram_tensor("wv", [128, 8 * QC], F16, kind="ExternalInput")
    bq_d = nc.dram_tensor("bq", [128, 2], F32, kind="ExternalInput")
    bk_d = nc.dram_tensor("bk", [128, 2], F32, kind="ExternalInput")
    bv_d = nc.dram_tensor("bv", [1, QC], F32, kind="ExternalInput")
    wp_d = nc.dram_tensor("wp", [128, 2 * C], F16, kind="ExternalInput")
    mask_d = nc.dram_tensor("mask", [128, 2 * 128], F16, kind="ExternalInput")
    yt_d = nc.dram_tensor("yt", [C, T], F16, kind="ExternalOutput")

    with tile.TileContext(nc) as tc:
        with tc.tile_pool(name="cst", bufs=1) as cst, \
             tc.tile_pool(name="wgt", bufs=1) as wgt, \
             tc.tile_pool(name="qk", bufs=1) as qkp, \
             tc.tile_pool(name="vv", bufs=1) as vvp, \
             tc.tile_pool(name="pp", bufs=6) as ppp, \
             tc.tile_pool(name="dn", bufs=3) as dnp, \
             tc.tile_pool(name="yy", bufs=4) as yyp, \
             tc.tile_pool(name="mm", bufs=2, space="PSUM") as mmp, \
             tc.tile_pool(name="ss", bufs=2, space="PSUM") as ssp, \
             tc.tile_pool(name="po", bufs=2, space="PSUM") as pop:

            # ---- constants / weights ----
            mask = cst.tile([128, 2, 128], F16, tag="mask")
            nc.gpsimd.dma_start(
                out=mask[:], in_=mask_d.ap().rearrange("p (a n) -> p a n", a=2))

            wq_s = wgt.tile([128, 4, 2, QC], F8, tag="wq")
            wk_s = wgt.tile([128, 4, 2, QC], F8, tag="wk")
            for wd, ws in ((wq_d, wq_s), (wk_d, wk_s)):
                nc.scalar.dma_start(
                    out=ws[:],
                    in_=wd.ap().rearrange("p (g j n) -> p g j n", g=4, j=2))
            wv_s = wgt.tile([128, 8, QC], F16, tag="wv")
            nc.scalar.dma_start(
                out=wv_s[:], in_=wv_d.ap().rearrange("p (c n) -> p c n", c=8))
            wp_s = wgt.tile([128, 2, C], F16, tag="wp")
            nc.scalar.dma_start(
                out=wp_s[:], in_=wp_d.ap().rearrange("p (j n) -> p j n", j=2))
            bq_s = cst.tile([128, 2], F32, tag="bq")
            bk_s = cst.tile([128, 2], F32, tag="bk")
            nc.gpsimd.dma_start(out=bq_s[:], in_=bq_d[:])
            nc.gpsimd.dma_start(out=bk_s[:], in_=bk_d[:])
            bv_row = cst.tile([1, QC], F32, tag="bvr")
            nc.gpsimd.dma_start(out=bv_row[:], in_=bv_d[:])
            bv_b = cst.tile([128, QC], F32, tag="bvb")
            nc.gpsimd.partition_broadcast(bv_b[:], bv_row[:])

            # X^T resident, chunk-major so each 512-t chunk is one
            # contiguous DMA; issues spread across engines so transfers
            # start immediately (serial DMA issue cost is ~1us each).
            xq_s = wgt.tile([128, NT4, 4, 2, 512], F8, tag="xq")
            xv_s = wgt.tile([128, NT4, 8, 512], F16, tag="xv")
            xq_r = xq_d.ap().rearrange(
                "p (t4 g j t) -> p t4 g j t", t4=NT4, g=4, j=2)
            xv_r = xv_d.ap().rearrange(
                "p (t4 c t) -> p t4 c t", t4=NT4, c=8)
            for t4 in range(NT4):
                nc.sync.dma_start(out=xq_s[:, t4], in_=xq_r[:, t4])
            for t4 in range(NT4):
                eng = nc.gpsimd if t4 < 2 else nc.sync
                eng.dma_start(out=xv_s[:, t4], in_=xv_r[:, t4])

            # ---- persistent activations ----
            qt_s = [qkp.tile([64, 2, T], F8, tag=f"qt{m}", name=f"qt{m}")
                    for m in range(2)]
            kt_s = [qkp.tile([64, 2, T], F8, tag=f"kt{m}", name=f"kt{m}")
                    for m in range(2)]
            # V': [128, kc, head, 64 v-cols + 64 ones-cols] f16
            vp_s = vvp.tile([128, NT, HPC, 2 * D], F16, tag="vp")
            nc.gpsimd.memset(vp_s[:, :, :, D:2 * D], 1.0)
            # O^T: [128, j, T] f16; head h at partitions (h%2)*64, j=h//2
            ot_s = qkp.tile([128, 2, T], F16, tag="ot")

            # ---------- emission helpers ----------
            def emit_qk(t4, m, ws, bs, dst):
                sl = slice(t4 * 512, (t4 + 1) * 512)
                pq = mmp.tile([128, 512], F32, tag="mm")
                for g in range(4):
                    nc.tensor.matmul(
                        pq[:], ws[:, g, :, m * 128:(m + 1) * 128],
                        xq_s[:, t4, g],
                        start=(g == 0), stop=(g == 3), perf_mode=DR)
                for j in range(2):
                    nc.vector.tensor_scalar_add(
                        dst[m][:, j, sl], pq[j * 64:(j + 1) * 64, :],
                        bs[j * 64:(j + 1) * 64, m:m + 1])

            def emit_v(t4, i):
                # two 128-t k-chunks (kc = 4*t4+2i, +1) share one PSUM tile
                pv = mmp.tile([128, 512], F32, tag="mm")
                for ii in range(2):
                    kk = 2 * i + ii
                    for c in range(8):
                        nc.tensor.matmul(
                            pv[:, ii * QC:(ii + 1) * QC],
                            xv_s[:, t4, c, kk * 128:(kk + 1) * 128],
                            wv_s[:, c, :],
                            start=(c == 0), stop=(c == 7))
                for ii in range(2):
                    kc = 4 * t4 + 2 * i + ii
                    nc.vector.tensor_tensor(
                        vp_s[:, kc, :, 0:D],
                        pv[:, ii * QC:(ii + 1) * QC].rearrange(
                            "p (h d) -> p h d", d=D),
                        bv_b[:].rearrange("p (h d) -> p h d", d=D),
                        op=ADD)

            def emit_proj(n4, mo, dma_eng):
                lo0, hi0 = n4 * 512, (n4 + 1) * 512
                py = mmp.tile([128, 512], F32, tag="mm")
                for j in range(2):
                    nc.tensor.matmul(
                        py[:], wp_s[:, j, mo * 128:(mo + 1) * 128],
                        ot_s[:, j, lo0:hi0],
                        start=(j == 0), stop=(j == 1))
                yt_stage = yyp.tile([128, 512], F16, tag="yt")
                nc.vector.tensor_copy(yt_stage[:], py[:])
                dma_eng.dma_start(
                    out=yt_d[mo * 128:(mo + 1) * 128, lo0:hi0], in_=yt_stage[:])

            def make_fills(t4):
                fills = []
                if t4 + 1 < NT4:
                    for m in range(2):
                        fills.append(lambda m=m: emit_qk(t4 + 1, m, wq_s, bq_s, qt_s))
                        fills.append(lambda m=m: emit_qk(t4 + 1, m, wk_s, bk_s, kt_s))
                    for i in range(2):
                        fills.append(lambda i=i: emit_v(t4 + 1, i))
                if t4 >= 1:
                    for mo in range(8):
                        eng = nc.sync if mo % 2 == 0 else nc.gpsimd
                        fills.append(lambda mo=mo, eng=eng: emit_proj(t4 - 1, mo, eng))
                return fills

            # attention group emitters.  A group is one ss PSUM tile:
            # either 2 off-diagonal k-chunks (full 512 cols each) or the
            # packed diagonal chunks (widths 512/384 or 256/128).
            def groups_of(t4):
                gs = []
                for g in range(2 * t4):
                    gs.append(("off", g))
                gs.append(("diag", 0))
                gs.append(("diag", 1))
                return gs

            def emit_S(t4, h, grp):
                lo0, hi0 = t4 * 512, (t4 + 1) * 512
                m, hh = h // 2, h % 2
                qt_h = qt_s[m][hh * 32:hh * 32 + 32, :, :]
                kt_h = kt_s[m][hh * 32:hh * 32 + 32, :, :]
                kind, g = grp
                sp = ssp.tile([128, 1024], F32, tag="ss")
                pt = ppp.tile([128, 1024], F16, tag="p")
                if kind == "off":
                    for jj in range(2):
                        kc = 2 * g + jj
                        nc.tensor.matmul(
                            sp[:, jj * 512:(jj + 1) * 512],
                            kt_h[:, :, kc * 128:(kc + 1) * 128],
                            qt_h[:, :, lo0:hi0],
                            start=True, stop=True, perf_mode=DR)
                    nc.scalar.activation(pt[:], sp[:], AF.Exp,
                                         scale=float(EXP_SCALE))
                    return (pt, None)
                else:
                    offs = []
                    pos = 0
                    for di in (2 * g, 2 * g + 1):
                        kc = 4 * t4 + di
                        lo = kc * 128
                        n = hi0 - lo
                        nc.tensor.matmul(
                            sp[:, pos:pos + n],
                            kt_h[:, :, kc * 128:(kc + 1) * 128],
                            qt_h[:, :, lo:hi0],
                            start=True, stop=True, perf_mode=DR)
                        offs.append((kc, lo, n, pos))
                        pos += n
                    nc.scalar.activation(pt[:, 0:pos], sp[:, 0:pos], AF.Exp,
                                         scale=float(EXP_SCALE))
                    # batched causal mask on the 2 diagonal squares (Pool):
                    # squares sit at columns 0 and o1 -> strided [128,2,128]
                    o1 = offs[1][3]
                    blocks = pt[:, 0:2 * o1].rearrange(
                        "p (a n) -> p a n", a=2)[:, :, 0:128]
                    nc.gpsimd.tensor_tensor(blocks, blocks, mask[:], op=MUL)
                    return (pt, offs)

            def emit_PV(t4, h, grp, pt_offs, op_tl, is_first, is_last):
                lo0 = t4 * 512
                kind, g = grp
                pt, offs = pt_offs
                if kind == "off":
                    for jj in range(2):
                        kc = 2 * g + jj
                        nc.tensor.matmul(
                            op_tl[:], vp_s[:, kc, h, :],
                            pt[:, jj * 512:(jj + 1) * 512],
                            start=(is_first and jj == 0), stop=False)
                else:
                    for idx, (kc, lo, n, off) in enumerate(offs):
                        nc.tensor.matmul(
                            op_tl[:, lo - lo0:512], vp_s[:, kc, h, :],
                            pt[:, off:off + n],
                            start=(is_first and idx == 0),
                            stop=(is_last and idx == 1))

            def emit_norm(t4, h, op_tl):
                m, hh = h // 2, h % 2
                sl = slice(t4 * 512, (t4 + 1) * 512)
                rc_in = dnp.tile([64, 512], F32, tag="rci")
                nc.vector.tensor_copy(rc_in[:], op_tl[D:2 * D, :])
                rc = dnp.tile([64, 512], F32, tag="rc")
                nc.vector.reciprocal_approx_fast(rc[:], rc_in[:])
                nc.vector.tensor_tensor(
                    ot_s[hh * 64:hh * 64 + 64, m, sl],
                    op_tl[0:D, :], rc[:], op=MUL)

            # ---------- main schedule ----------
            # round 0 lead-in
            for m in range(2):
                emit_qk(0, m, wq_s, bq_s, qt_s)
                emit_qk(0, m, wk_s, bk_s, kt_s)
            for i in range(2):
                emit_v(0, i)

            for t4 in range(NT4):
                fills = make_fills(t4)
                fi = 0

                def fill():
                    nonlocal fi
                    if fi < len(fills):
                        fills[fi]()
                        fi += 1

                gs = groups_of(t4)
                ng = len(gs)
                for pair in range(2):
                    hA, hB = 2 * pair, 2 * pair + 1
                    opA = pop.tile([128, 512], F32, tag="po", name=f"opA{t4}_{pair}")
                    opB = pop.tile([128, 512], F32, tag="po", name=f"opB{t4}_{pair}")
                    pts = {}
                    # prime the pipeline 1 group deep per head
                    pts[(hA, 0)] = emit_S(t4, hA, gs[0])
                    pts[(hB, 0)] = emit_S(t4, hB, gs[0])
                    for g in range(ng):
                        for h, op_tl in ((hA, opA), (hB, opB)):
                            emit_PV(t4, h, gs[g], pts.pop((h, g)), op_tl,
                                    is_first=(g == 0), is_last=(g == ng - 1))
                            if g + 1 < ng:
                                pts[(h, g + 1)] = emit_S(t4, h, gs[g + 1])
                            fill()
                    emit_norm(t4, hA, opA)
                    emit_norm(t4, hB, opB)
                    fill()
                # drain remaining fills
                while fi < len(fills):
                    fills[fi]()
                    fi += 1

            # tail: projection for the final round
            for mo in range(8):
                eng = nc.sync if mo % 2 == 0 else nc.gpsimd
                emit_proj(NT4 - 1, mo, eng)

    nc.finalize()
    return nc


def _get_program():
    if "nc" not in _cache:
        _ensure_env()
        _cache["nc"] = _build()
    return _cache["nc"]


def _qkv_perm():
    """Column permutation within a core's 256 q (or k) columns.

    New position m*128 + j*64 + hh*32 + dd holds original column
    (2m+hh)*64 + j*32 + dd  (m = head-pair, hh = head in pair,
    j = d//32, dd = d%32).
    """
    perm = np.empty(QC, dtype=np.int64)
    pos = 0
    for m in range(2):
        for j in range(2):
            for hh in range(2):
                for dd in range(32):
                    perm[pos] = (2 * m + hh) * 64 + j * 32 + dd
                    pos += 1
    return perm


def kernel(x, w_attn, b_attn, w_proj, b_proj):
    import ml_dtypes
    F8 = ml_dtypes.float8_e4m3

    x = np.ascontiguousarray(np.asarray(x, dtype=np.float32))
    w_attn = np.ascontiguousarray(np.asarray(w_attn, dtype=np.float32))
    b_attn = np.ascontiguousarray(np.asarray(b_attn, dtype=np.float32))
    w_proj = np.ascontiguousarray(np.asarray(w_proj, dtype=np.float32))
    b_proj = np.ascontiguousarray(np.asarray(b_proj, dtype=np.float32))

    nc = _get_program()
    from concourse.bass_utils import run_bass_kernel_spmd

    tri = np.triu(np.ones((128, 128), dtype=np.float32)).astype(np.float16)
    mask2 = np.ascontiguousarray(
        np.concatenate([tri, tri], axis=1))  # [128, 2*128]
    perm = _qkv_perm()

    xq_all, xv_all = [], []
    for b in range(B):
        xt = x[b].T.reshape(8, 128, T).transpose(1, 0, 2)  # [128, c, T]
        # chunk-major: [128, t4, c, 512]
        xt4 = xt.reshape(128, 8, 4, 512).transpose(0, 2, 1, 3)
        xq_all.append(np.ascontiguousarray(
            xt4.astype(F8).reshape(128, 8 * T)))
        xv_all.append(np.ascontiguousarray(
            xt4.astype(np.float16).reshape(128, 8 * T)))

    in_maps = []
    for c in range(NCORES):
        b = c // 4
        hg = c % 4
        q0 = hg * QC

        def wqk8(off):
            w = (WS * w_attn[:, off + q0:off + q0 + QC])[:, perm]
            w8 = w.astype(F8).reshape(4, 2, 128, QC).transpose(2, 0, 1, 3)
            return np.ascontiguousarray(w8.reshape(128, 8 * QC))

        def bqk(off):
            bb = (WS * b_attn[off + q0:off + q0 + QC])[perm]
            return np.ascontiguousarray(bb.reshape(2, 128).T.astype(np.float32))

        wv = w_attn[:, 2 * C + q0:2 * C + q0 + QC].astype(np.float16)
        wv = wv.reshape(8, 128, QC).transpose(1, 0, 2)
        wp = w_proj[q0:q0 + QC, :].astype(np.float16)
        wp = wp.reshape(2, 128, C).transpose(1, 0, 2)

        in_maps.append({
            "xq": xq_all[b],
            "xv": xv_all[b],
            "wq": wqk8(0),
            "wk": wqk8(C),
            "wv": np.ascontiguousarray(wv.reshape(128, 8 * QC)),
            "bq": bqk(0),
            "bk": bqk(C),
            "bv": np.ascontiguousarray(
                b_attn[2 * C + q0:2 * C + q0 + QC].reshape(1, QC)
                .astype(np.float32)),
            "wp": np.ascontiguousarray(wp.reshape(128, 2 * C)),
            "mask": mask2,
        })

    trace = bool(os.environ.get("KERNEL_TRACE"))
    res = run_bass_kernel_spmd(nc, in_maps, list(range(NCORES)), trace=trace)
    _cache["last_results"] = res

    out = np.empty((B, T, C), dtype=np.float32)
    for b in range(B):
        acc = res.results[4 * b]["yt"].astype(np.float32)
        for c in range(4 * b + 1, 4 * b + 4):
            acc = acc + res.results[c]["yt"].astype(np.float32)
        out[b] = acc.T + b_proj
    return out
